# revision 21
# baseline (speedup 1.0000x reference)
"""DeepSATConv GNN message-passing kernel for 8 Trainium2 NeuronCores.

Math note: the reference computes a per-channel segment-softmax over
msg = self_h[src] + neib_h[dst].  Within a dst-segment, neib_h[dst] (and
b_self, b_nb) are constant per channel, so they cancel in the softmax.
Hence alpha = segsoftmax(h @ W_self.T) exactly, and
out[n] = segsum(e * h[src]) / segsum(e)  with e = exp((h @ W_self.T)[src]),
falling back to h[n] for zero-in-degree nodes.  W_nb / b_nb / b_self do
not affect the output at all.

Sharding: nodes are split across the 8 cores (2500 each); edges are
partitioned by destination node so segment reductions stay core-local;
h is replicated (the "halo gather" degenerates to replication).

Design notes (evidence from NTFF traces):
- everything feeding the PE is bf16 (4x the fp32 matmul rate),
- the gathered row packs [e | h] bf16 so ONE dma_gather descriptor per
  edge fetches both operands (descriptor GENERATION on the Q7 cores is
  the hard bottleneck: ~6.8 ns/descriptor + ~1.9 us/call, so calls are
  merged two node-tiles at a time),
- Z's h-columns are pre-filled by the host (Z is an ExternalInput); the
  device computes only the e-columns (2 bf16 matmuls per 128-node tile,
  one Exp activation per 4 tiles, batched strided writes),
- the one-hot selector S[e, n] = (dst_local[e] == n) is built on the
  host and DMA'd in bf16 (building it on the DVE made the gather stream
  stall on the DVE's instruction-counter semaphore),
- S loads and gathers are issued PF groups ahead of consumption so the
  gather stream never waits on buffers; the last pair is split into two
  single-tile calls so the tail overlaps the final gather,
- finalize reads the PSUM accumulator directly and uses the fast
  approximate reciprocal.

Numerics: bf16 tables + bf16 selector matmul + W_hi-only phase A give
~3.9e-3 relative error vs the 2e-2 budget (validated on HW).
"""

import numpy as np

N_NODES = 20000
N_EDGES = 320000
D = 256
CORES = 8
NPC = N_NODES // CORES          # 2500 nodes per core
NT = (NPC + 127) // 128         # 20 node tiles per core
NROWS = NT * 128                # 2560 padded rows per core
NT_ALL = 160                    # phase-A 128-node tiles over all nodes
NPAD = NT_ALL * 128             # 20480
QT = 4                          # phase-A tiles per PSUM group
HQ = 2                          # hT quads per DMA
GT = 2                          # node-tiles per dma_gather call
BB = 6                          # chunks per DVE mult batch

_cache = {}


def _build(caps):
    import concourse.bacc as bacc
    import concourse.mybir as mybir
    from concourse.tile import TileContext

    nc = bacc.Bacc("TRN2")
    f32 = mybir.dt.float32
    bf16 = mybir.dt.bfloat16

    NCH = sum(caps)                     # total chunks across tiles
    NIX = 128 * NCH                     # total gathered edge slots
    GMAX = max(
        sum(caps[t0:t0 + GT]) for t0 in range(0, NT, GT)
    )
    CTMAX = max(caps)

    hT_d = nc.dram_tensor("hT", [128, 2, NPAD], bf16, kind="ExternalInput")
    WT_d = nc.dram_tensor("WT", [128, 2, D], bf16, kind="ExternalInput")
    Z_d = nc.dram_tensor("Z", [NPAD, 2 * D], bf16, kind="ExternalInput")
    idx_d = nc.dram_tensor("idx", [128, NIX // 16], mybir.dt.int16, kind="ExternalInput")
    S_d = nc.dram_tensor("S", [128, NCH, 128], bf16, kind="ExternalInput")
    hown_d = nc.dram_tensor("hown", [NROWS, D], bf16, kind="ExternalInput")
    out_d = nc.dram_tensor("out", [NROWS, D], bf16, kind="ExternalOutput")

    with TileContext(nc) as tc:
        with (
            tc.tile_pool(name="const", bufs=1) as constp,
            tc.tile_pool(name="pha", bufs=3) as pha,
            tc.tile_pool(name="phb", bufs=3) as phb,
            tc.tile_pool(name="gat", bufs=3) as gat,
            tc.tile_pool(name="wrk", bufs=3) as wrk,
            tc.tile_pool(name="fin", bufs=2) as fin,
            tc.tile_pool(name="psa", bufs=2, space="PSUM") as psa,
            tc.tile_pool(name="psb", bufs=3, space="PSUM") as psb,
        ):
            # ---- constants ----
            WT_sb = constp.tile([128, 2, D], bf16)
            nc.sync.dma_start(WT_sb[:, :, :], WT_d[:, :, :])
            idx_sb = constp.tile([128, NIX // 16], mybir.dt.int16)

            # ---- phase A: e-columns of Z ----
            for i in range(NT_ALL // (QT * HQ)):
                hT_sb = phb.tile([128, 2, QT * HQ * 128], bf16, tag="hT")
                nc.sync.dma_start(
                    hT_sb[:, :, :],
                    hT_d[:, :, i * QT * HQ * 128:(i + 1) * QT * HQ * 128],
                )
                e_sb = pha.tile([128, HQ, QT, D], bf16, tag="es")
                for q in range(HQ):
                    ps = psa.tile([128, QT, D], f32, tag="ps")
                    for u in range(QT):
                        for kb in range(2):
                            nc.tensor.matmul(
                                ps[:, u, :],
                                hT_sb[:, kb, (q * QT + u) * 128:(q * QT + u + 1) * 128],
                                WT_sb[:, kb, :],
                                start=(kb == 0), stop=(kb == 1),
                            )
                    nc.scalar.activation(
                        e_sb[:, q, :, :], ps[:, :, :],
                        mybir.ActivationFunctionType.Exp,
                    )
                r0 = i * HQ * QT * 128
                zrows = Z_d[r0:r0 + HQ * QT * 128, 0:D]
                nc.sync.dma_start(
                    zrows.rearrange("(q u p) c -> p q u c", p=128, q=HQ),
                    e_sb[:, :, :, :],
                )

            # idx for the gathers (loaded on the scalar queue so the sync
            # queue drains the final Z e-writes without queueing behind it)
            nc.scalar.dma_start(idx_sb[:, :], idx_d[:, :])

            # ---- phase B: gathers (2 tiles per call) + segment softmax ----
            # Pipeline: S-selectors are built (DVE) and gathers issued
            # (gpsimd) PF pairs ahead of consumption, so the ehx mult never
            # head-blocks the DVE queue on an in-flight gather and the
            # gather stream never waits on zx-buffer reuse.
            coffs = []
            co = 0
            for t in range(NT):
                coffs.append(co)
                co += caps[t]
            # gather call groups: pairs, except the last pair is split so
            # the tail consumption overlaps the final (small) gather
            groups = [(t0, GT) for t0 in range(0, NT - GT, GT)]
            groups += [(NT - GT, 1), (NT - 1, 1)]
            NP_ = len(groups)
            PF = 2                      # gather prefetch distance (groups)
            zxs = {}
            Ss = {}

            def emit_sload(tp):
                t0, nt_ = groups[tp]
                C01 = sum(caps[t0:t0 + nt_])
                co0 = coffs[t0]
                S_p = wrk.tile([128, GMAX, 128], bf16, tag="S")
                nc.scalar.dma_start(
                    S_p[:, 0:C01, :], S_d[:, co0:co0 + C01, :]
                )
                Ss[tp] = S_p

            def emit_gather(tp):
                t0, nt_ = groups[tp]
                C01 = sum(caps[t0:t0 + nt_])
                co0 = coffs[t0]
                zx = gat.tile([128, GMAX, 2 * D], bf16, tag="zx")
                nc.gpsimd.dma_gather(
                    zx[:, 0:C01, :], Z_d[:, :],
                    idx_sb[:, co0 * 8:(co0 + C01) * 8], 128 * C01, 128 * C01,
                    2 * D, single_packet=False,
                )
                zxs[tp] = zx

            for tp in range(PF):
                emit_sload(tp)
                emit_gather(tp)

            for tp in range(NP_):
                if tp + PF < NP_:
                    emit_sload(tp + PF)
                    emit_gather(tp + PF)
                zx = zxs.pop(tp)
                S_p = Ss.pop(tp)
                g0, gn = groups[tp]
                co0 = coffs[g0]
                for tt in range(gn):
                    t = g0 + tt
                    C = caps[t]
                    zo = coffs[t] - co0     # chunk offset inside zx
                    S_t = S_p
                    ehx = wrk.tile([128, CTMAX, D], bf16, tag="ehx")
                    for g in range((C + BB - 1) // BB):
                        b = min(BB, C - g * BB)
                        nc.vector.tensor_tensor(
                            ehx[:, g * BB:g * BB + b, :],
                            zx[:, zo + g * BB:zo + g * BB + b, 0:D],
                            zx[:, zo + g * BB:zo + g * BB + b, D:2 * D],
                            mybir.AluOpType.mult,
                        )
                    acc = psb.tile([128, 2 * D], f32, tag="acc")
                    for j in range(C):
                        nc.tensor.matmul(
                            acc[:, 0:D], S_t[:, zo + j, :], zx[:, zo + j, 0:D],
                            start=(j == 0), stop=(j == C - 1),
                        )
                    for j in range(C):
                        nc.tensor.matmul(
                            acc[:, D:2 * D], S_t[:, zo + j, :], ehx[:, j, :],
                            start=(j == 0), stop=(j == C - 1),
                        )

                    # ---- finalize tile (reads PSUM directly) ----
                    dmax = fin.tile([128, D], f32, tag="dmax")
                    nc.vector.tensor_scalar(
                        dmax[:, :], acc[:, 0:D], 1e-30, None, mybir.AluOpType.max
                    )
                    rec = fin.tile([128, D], f32, tag="rec")
                    nc.vector.reciprocal_approx_fast(rec[:, :], dmax[:, :])
                    mask = fin.tile([128, D], mybir.dt.uint8, tag="mask")
                    nc.vector.tensor_scalar(
                        mask[:, :], acc[:, 0:D], 0.0, None, mybir.AluOpType.is_equal
                    )
                    res = fin.tile([128, D], bf16, tag="res")
                    nc.vector.tensor_tensor(
                        res[:, :], acc[:, D:2 * D], rec[:, :], mybir.AluOpType.mult
                    )
                    hown_sb = fin.tile([128, D], bf16, tag="hown")
                    nc.scalar.dma_start(
                        hown_sb[:, :], hown_d[t * 128:(t + 1) * 128, :]
                    )
                    nc.vector.copy_predicated(res[:, :], mask[:, :], hown_sb[:, :])
                    nc.sync.dma_start(out_d[t * 128:(t + 1) * 128, :], res[:, :])
    nc.compile()
    return nc


def _wrap_idx(ix):
    # dma_gather index layout: logical index i lands at output
    # [partition i%128, slot i//128]; the SBUF index tile stores it at
    # [i%16, 8*(i//128) + (i%128)//16], replicated over the 8 Q7 cores.
    w = ix.astype(np.int16).reshape(-1, 8, 16).transpose(2, 0, 1).reshape(16, -1)
    return np.tile(w, (8, 1))


def kernel(h, W_nb, b_nb, W_self, b_self, src, dst):
    from concourse.bass_utils import run_bass_kernel_spmd
    import ml_dtypes

    bf = ml_dtypes.bfloat16
    h = np.ascontiguousarray(np.asarray(h, dtype=np.float32))
    W = np.asarray(W_self, dtype=np.float32)
    src = np.asarray(src, dtype=np.int64)
    dst = np.asarray(dst, dtype=np.int64)

    order = np.argsort(dst, kind="stable")
    src_s = src[order]
    dst_s = dst[order]

    # per-(core, tile) edge ranges; tiles are 128 consecutive owned nodes
    tile_base = []
    for c in range(CORES):
        for t in range(NT):
            tile_base.append(c * NPC + t * 128)
    bounds_lo = np.searchsorted(dst_s, np.array(tile_base), side="left")
    hi_nodes = [min(b + 128, (b // NPC + 1) * NPC) for b in tile_base]
    bounds_hi = np.searchsorted(dst_s, np.array(hi_nodes), side="left")

    cnt = np.zeros((CORES, NT), dtype=np.int64)
    for c in range(CORES):
        for t in range(NT):
            cnt[c, t] = bounds_hi[c * NT + t] - bounds_lo[c * NT + t]
    caps = [int((cnt[:, t].max() + 127) // 128) for t in range(NT)]
    assert max(caps[t] + caps[t + 1] for t in range(0, NT, GT)) <= 40, caps
    NCH = sum(caps)

    # host-side layout prep
    h_bf = h.astype(bf)
    hT = np.zeros((128, 2, NPAD), dtype=bf)
    hT[:, :, :N_NODES] = np.ascontiguousarray(
        h_bf.T.reshape(2, 128, N_NODES).transpose(1, 0, 2)
    )
    WT = np.ascontiguousarray(
        W.astype(bf).T.reshape(2, 128, D).transpose(1, 0, 2)
    )
    Z = np.zeros((NPAD, 2 * D), dtype=bf)
    Z[:N_NODES, D:2 * D] = h_bf

    in_maps = []
    for c in range(CORES):
        idx_parts = []
        S_all = np.zeros((128, NCH, 128), dtype=bf)
        coff = 0
        for t in range(NT):
            Ct = caps[t]
            CAPs = 128 * Ct
            i = c * NT + t
            lo, hi = int(bounds_lo[i]), int(bounds_hi[i])
            n = hi - lo
            spad = np.zeros(CAPs, dtype=np.int64)
            spad[:n] = src_s[lo:hi]
            idx_parts.append(_wrap_idx(spad))
            ei = np.arange(n)
            S_all[ei % 128, coff + ei // 128, dst_s[lo:hi] - tile_base[i]] = 1.0
            coff += Ct
        hown = np.zeros((NROWS, D), dtype=bf)
        hown[:NPC] = h_bf[c * NPC:(c + 1) * NPC]
        in_maps.append({
            "hT": hT,
            "WT": WT,
            "Z": Z,
            "idx": np.ascontiguousarray(np.concatenate(idx_parts, axis=1)),
            "S": S_all,
            "hown": hown,
        })

    key = tuple(caps)
    if key not in _cache:
        _cache[key] = _build(caps)
    nc = _cache[key]

    res = run_bass_kernel_spmd(nc, in_maps, core_ids=list(range(CORES)))
    out = np.concatenate(
        [res.results[c]["out"][:NPC] for c in range(CORES)], axis=0
    )
    return out.astype(np.float32)


# revision 22
# speedup vs baseline: 1.0041x; 1.0041x over previous
"""DeepSATConv GNN message-passing kernel for 8 Trainium2 NeuronCores.

Math note: the reference computes a per-channel segment-softmax over
msg = self_h[src] + neib_h[dst].  Within a dst-segment, neib_h[dst] (and
b_self, b_nb) are constant per channel, so they cancel in the softmax.
Hence alpha = segsoftmax(h @ W_self.T) exactly, and
out[n] = segsum(e * h[src]) / segsum(e)  with e = exp((h @ W_self.T)[src]),
falling back to h[n] for zero-in-degree nodes.  W_nb / b_nb / b_self do
not affect the output at all.

Sharding: nodes are split across the 8 cores (2500 each); edges are
partitioned by destination node so segment reductions stay core-local;
h is replicated (the "halo gather" degenerates to replication).

Design notes (evidence from NTFF traces):
- everything feeding the PE is bf16 (4x the fp32 matmul rate),
- the gathered row packs [e | h] bf16 so ONE dma_gather descriptor per
  edge fetches both operands (descriptor GENERATION on the Q7 cores is
  the hard bottleneck: ~6.8 ns/descriptor + ~1.9 us/call, so calls are
  merged two node-tiles at a time),
- Z's h-columns are pre-filled by the host (Z is an ExternalInput); the
  device computes only the e-columns (2 bf16 matmuls per 128-node tile,
  one Exp activation per 4 tiles, batched strided writes),
- the one-hot selector S[e, n] = (dst_local[e] == n) is built on the
  host and DMA'd in bf16 (building it on the DVE made the gather stream
  stall on the DVE's instruction-counter semaphore),
- S loads and gathers are issued PF groups ahead of consumption so the
  gather stream never waits on buffers; the last pair is split into two
  single-tile calls so the tail overlaps the final gather,
- finalize reads the PSUM accumulator directly and uses the fast
  approximate reciprocal.

Numerics: bf16 tables + bf16 selector matmul + W_hi-only phase A give
~3.9e-3 relative error vs the 2e-2 budget (validated on HW).
"""

import numpy as np

N_NODES = 20000
N_EDGES = 320000
D = 256
CORES = 8
NPC = N_NODES // CORES          # 2500 nodes per core
NT = (NPC + 127) // 128         # 20 node tiles per core
NROWS = NT * 128                # 2560 padded rows per core
NT_ALL = 160                    # phase-A 128-node tiles over all nodes
NPAD = NT_ALL * 128             # 20480
QT = 4                          # phase-A tiles per PSUM group
HQ = 2                          # hT quads per DMA
GT = 2                          # node-tiles per dma_gather call
BB = 6                          # chunks per DVE mult batch

_cache = {}


def _build(caps):
    import concourse.bacc as bacc
    import concourse.mybir as mybir
    from concourse.tile import TileContext

    nc = bacc.Bacc("TRN2")
    f32 = mybir.dt.float32
    bf16 = mybir.dt.bfloat16

    NCH = sum(caps)                     # total chunks across tiles
    NIX = 128 * NCH                     # total gathered edge slots
    GMAX = max(
        sum(caps[t0:t0 + GT]) for t0 in range(0, NT, GT)
    )
    CTMAX = max(caps)

    hT_d = nc.dram_tensor("hT", [128, 2, NPAD], bf16, kind="ExternalInput")
    WT_d = nc.dram_tensor("WT", [128, 2, D], bf16, kind="ExternalInput")
    Z_d = nc.dram_tensor("Z", [NPAD, 2 * D], bf16, kind="ExternalInput")
    idx_d = nc.dram_tensor("idx", [128, NIX // 16], mybir.dt.int16, kind="ExternalInput")
    S_d = nc.dram_tensor("S", [128, NCH, 128], bf16, kind="ExternalInput")
    hown_d = nc.dram_tensor("hown", [NROWS, D], bf16, kind="ExternalInput")
    out_d = nc.dram_tensor("out", [NROWS, D], bf16, kind="ExternalOutput")

    with TileContext(nc) as tc:
        with (
            tc.tile_pool(name="const", bufs=1) as constp,
            tc.tile_pool(name="pha", bufs=3) as pha,
            tc.tile_pool(name="phb", bufs=3) as phb,
            tc.tile_pool(name="gat", bufs=3) as gat,
            tc.tile_pool(name="wrk", bufs=3) as wrk,
            tc.tile_pool(name="fin", bufs=2) as fin,
            tc.tile_pool(name="psa", bufs=2, space="PSUM") as psa,
            tc.tile_pool(name="psb", bufs=3, space="PSUM") as psb,
        ):
            # ---- constants ----
            WT_sb = constp.tile([128, 2, D], bf16)
            nc.sync.dma_start(WT_sb[:, :, :], WT_d[:, :, :])
            idx_sb = constp.tile([128, NIX // 16], mybir.dt.int16)

            # ---- phase A: e-columns of Z ----
            for i in range(NT_ALL // (QT * HQ)):
                hT_sb = phb.tile([128, 2, QT * HQ * 128], bf16, tag="hT")
                nc.sync.dma_start(
                    hT_sb[:, :, :],
                    hT_d[:, :, i * QT * HQ * 128:(i + 1) * QT * HQ * 128],
                )
                e_sb = pha.tile([128, HQ, QT, D], bf16, tag="es")
                for q in range(HQ):
                    ps = psa.tile([128, QT, D], f32, tag="ps")
                    for u in range(QT):
                        for kb in range(2):
                            nc.tensor.matmul(
                                ps[:, u, :],
                                hT_sb[:, kb, (q * QT + u) * 128:(q * QT + u + 1) * 128],
                                WT_sb[:, kb, :],
                                start=(kb == 0), stop=(kb == 1),
                            )
                    nc.scalar.activation(
                        e_sb[:, q, :, :], ps[:, :, :],
                        mybir.ActivationFunctionType.Exp,
                    )
                for q in range(HQ):
                    r0 = (i * HQ + q) * QT * 128
                    zrows = Z_d[r0:r0 + QT * 128, 0:D]
                    nc.sync.dma_start(
                        zrows.rearrange("(u p) c -> p u c", p=128),
                        e_sb[:, q, :, :],
                    )

            # idx for the gathers (loaded on the scalar queue so the sync
            # queue drains the final Z e-writes without queueing behind it)
            nc.scalar.dma_start(idx_sb[:, :], idx_d[:, :])

            # ---- phase B: gathers (2 tiles per call) + segment softmax ----
            # Pipeline: S-selectors are built (DVE) and gathers issued
            # (gpsimd) PF pairs ahead of consumption, so the ehx mult never
            # head-blocks the DVE queue on an in-flight gather and the
            # gather stream never waits on zx-buffer reuse.
            coffs = []
            co = 0
            for t in range(NT):
                coffs.append(co)
                co += caps[t]
            # gather call groups: pairs, except the last pair is split so
            # the tail consumption overlaps the final (small) gather
            groups = [(t0, GT) for t0 in range(0, NT - GT, GT)]
            groups += [(NT - GT, 1), (NT - 1, 1)]
            NP_ = len(groups)
            PF = 2                      # gather prefetch distance (groups)
            zxs = {}
            Ss = {}

            def emit_sload(tp):
                t0, nt_ = groups[tp]
                C01 = sum(caps[t0:t0 + nt_])
                co0 = coffs[t0]
                S_p = wrk.tile([128, GMAX, 128], bf16, tag="S")
                nc.scalar.dma_start(
                    S_p[:, 0:C01, :], S_d[:, co0:co0 + C01, :]
                )
                Ss[tp] = S_p

            def emit_gather(tp):
                t0, nt_ = groups[tp]
                C01 = sum(caps[t0:t0 + nt_])
                co0 = coffs[t0]
                zx = gat.tile([128, GMAX, 2 * D], bf16, tag="zx")
                # the very last call is split in half so the tail's matmuls
                # overlap the final drain (slice-level deps allow it)
                parts = [(0, C01)] if tp != NP_ - 1 else [
                    (0, C01 // 2), (C01 // 2, C01 - C01 // 2)
                ]
                for po, pc in parts:
                    nc.gpsimd.dma_gather(
                        zx[:, po:po + pc, :], Z_d[:, :],
                        idx_sb[:, (co0 + po) * 8:(co0 + po + pc) * 8],
                        128 * pc, 128 * pc, 2 * D, single_packet=False,
                    )
                zxs[tp] = zx

            for tp in range(PF):
                emit_sload(tp)
                emit_gather(tp)

            for tp in range(NP_):
                if tp + PF < NP_:
                    emit_sload(tp + PF)
                    emit_gather(tp + PF)
                zx = zxs.pop(tp)
                S_p = Ss.pop(tp)
                g0, gn = groups[tp]
                co0 = coffs[g0]
                for tt in range(gn):
                    t = g0 + tt
                    C = caps[t]
                    zo = coffs[t] - co0     # chunk offset inside zx
                    S_t = S_p
                    ehx = wrk.tile([128, CTMAX, D], bf16, tag="ehx")
                    for g in range((C + BB - 1) // BB):
                        b = min(BB, C - g * BB)
                        nc.vector.tensor_tensor(
                            ehx[:, g * BB:g * BB + b, :],
                            zx[:, zo + g * BB:zo + g * BB + b, 0:D],
                            zx[:, zo + g * BB:zo + g * BB + b, D:2 * D],
                            mybir.AluOpType.mult,
                        )
                    acc = psb.tile([128, 2 * D], f32, tag="acc")
                    for j in range(C):
                        nc.tensor.matmul(
                            acc[:, 0:D], S_t[:, zo + j, :], zx[:, zo + j, 0:D],
                            start=(j == 0), stop=(j == C - 1),
                        )
                    for j in range(C):
                        nc.tensor.matmul(
                            acc[:, D:2 * D], S_t[:, zo + j, :], ehx[:, j, :],
                            start=(j == 0), stop=(j == C - 1),
                        )

                    # ---- finalize tile (reads PSUM directly) ----
                    dmax = fin.tile([128, D], f32, tag="dmax")
                    nc.vector.tensor_scalar(
                        dmax[:, :], acc[:, 0:D], 1e-30, None, mybir.AluOpType.max
                    )
                    rec = fin.tile([128, D], f32, tag="rec")
                    nc.vector.reciprocal_approx_fast(rec[:, :], dmax[:, :])
                    mask = fin.tile([128, D], mybir.dt.uint8, tag="mask")
                    nc.vector.tensor_scalar(
                        mask[:, :], acc[:, 0:D], 0.0, None, mybir.AluOpType.is_equal
                    )
                    res = fin.tile([128, D], bf16, tag="res")
                    nc.vector.tensor_tensor(
                        res[:, :], acc[:, D:2 * D], rec[:, :], mybir.AluOpType.mult
                    )
                    hown_sb = fin.tile([128, D], bf16, tag="hown")
                    nc.scalar.dma_start(
                        hown_sb[:, :], hown_d[t * 128:(t + 1) * 128, :]
                    )
                    nc.vector.copy_predicated(res[:, :], mask[:, :], hown_sb[:, :])
                    nc.sync.dma_start(out_d[t * 128:(t + 1) * 128, :], res[:, :])
    nc.compile()
    return nc


def _wrap_idx(ix):
    # dma_gather index layout: logical index i lands at output
    # [partition i%128, slot i//128]; the SBUF index tile stores it at
    # [i%16, 8*(i//128) + (i%128)//16], replicated over the 8 Q7 cores.
    w = ix.astype(np.int16).reshape(-1, 8, 16).transpose(2, 0, 1).reshape(16, -1)
    return np.tile(w, (8, 1))


def kernel(h, W_nb, b_nb, W_self, b_self, src, dst):
    from concourse.bass_utils import run_bass_kernel_spmd
    import ml_dtypes

    bf = ml_dtypes.bfloat16
    h = np.ascontiguousarray(np.asarray(h, dtype=np.float32))
    W = np.asarray(W_self, dtype=np.float32)
    src = np.asarray(src, dtype=np.int64)
    dst = np.asarray(dst, dtype=np.int64)

    order = np.argsort(dst, kind="stable")
    src_s = src[order]
    dst_s = dst[order]

    # per-(core, tile) edge ranges; tiles are 128 consecutive owned nodes
    tile_base = []
    for c in range(CORES):
        for t in range(NT):
            tile_base.append(c * NPC + t * 128)
    bounds_lo = np.searchsorted(dst_s, np.array(tile_base), side="left")
    hi_nodes = [min(b + 128, (b // NPC + 1) * NPC) for b in tile_base]
    bounds_hi = np.searchsorted(dst_s, np.array(hi_nodes), side="left")

    cnt = np.zeros((CORES, NT), dtype=np.int64)
    for c in range(CORES):
        for t in range(NT):
            cnt[c, t] = bounds_hi[c * NT + t] - bounds_lo[c * NT + t]
    caps = [int((cnt[:, t].max() + 127) // 128) for t in range(NT)]
    assert max(caps[t] + caps[t + 1] for t in range(0, NT, GT)) <= 40, caps
    NCH = sum(caps)

    # host-side layout prep
    h_bf = h.astype(bf)
    hT = np.zeros((128, 2, NPAD), dtype=bf)
    hT[:, :, :N_NODES] = np.ascontiguousarray(
        h_bf.T.reshape(2, 128, N_NODES).transpose(1, 0, 2)
    )
    WT = np.ascontiguousarray(
        W.astype(bf).T.reshape(2, 128, D).transpose(1, 0, 2)
    )
    Z = np.zeros((NPAD, 2 * D), dtype=bf)
    Z[:N_NODES, D:2 * D] = h_bf

    in_maps = []
    for c in range(CORES):
        idx_parts = []
        S_all = np.zeros((128, NCH, 128), dtype=bf)
        coff = 0
        for t in range(NT):
            Ct = caps[t]
            CAPs = 128 * Ct
            i = c * NT + t
            lo, hi = int(bounds_lo[i]), int(bounds_hi[i])
            n = hi - lo
            spad = np.zeros(CAPs, dtype=np.int64)
            spad[:n] = src_s[lo:hi]
            idx_parts.append(_wrap_idx(spad))
            ei = np.arange(n)
            S_all[ei % 128, coff + ei // 128, dst_s[lo:hi] - tile_base[i]] = 1.0
            coff += Ct
        hown = np.zeros((NROWS, D), dtype=bf)
        hown[:NPC] = h_bf[c * NPC:(c + 1) * NPC]
        in_maps.append({
            "hT": hT,
            "WT": WT,
            "Z": Z,
            "idx": np.ascontiguousarray(np.concatenate(idx_parts, axis=1)),
            "S": S_all,
            "hown": hown,
        })

    key = tuple(caps)
    if key not in _cache:
        _cache[key] = _build(caps)
    nc = _cache[key]

    res = run_bass_kernel_spmd(nc, in_maps, core_ids=list(range(CORES)))
    out = np.concatenate(
        [res.results[c]["out"][:NPC] for c in range(CORES)], axis=0
    )
    return out.astype(np.float32)


# revision 25
# speedup vs baseline: 1.0158x; 1.0117x over previous
"""DeepSATConv GNN message-passing kernel for 8 Trainium2 NeuronCores.

Math note: the reference computes a per-channel segment-softmax over
msg = self_h[src] + neib_h[dst].  Within a dst-segment, neib_h[dst] (and
b_self, b_nb) are constant per channel, so they cancel in the softmax.
Hence alpha = segsoftmax(h @ W_self.T) exactly, and
out[n] = segsum(e * h[src]) / segsum(e)  with e = exp((h @ W_self.T)[src]),
falling back to h[n] for zero-in-degree nodes.  W_nb / b_nb / b_self do
not affect the output at all.

Sharding: nodes are split across the 8 cores (2500 each); edges are
partitioned by destination node so segment reductions stay core-local;
h is replicated (the "halo gather" degenerates to replication).

Design notes (evidence from NTFF traces):
- everything feeding the PE is bf16 (4x the fp32 matmul rate),
- the gathered row packs [e | h] bf16 so ONE dma_gather descriptor per
  edge fetches both operands (descriptor GENERATION on the Q7 cores is
  the hard bottleneck: ~6.8 ns/descriptor + ~1.9 us/call, so calls are
  merged two node-tiles at a time),
- Z's h-columns are pre-filled by the host (Z is an ExternalInput); the
  device computes only the e-columns (2 bf16 matmuls per 128-node tile,
  one Exp activation per 4 tiles, batched strided writes),
- the one-hot selector S[e, n] = (dst_local[e] == n) is built on the
  host and DMA'd in bf16 (building it on the DVE made the gather stream
  stall on the DVE's instruction-counter semaphore),
- S loads and gathers are issued PF groups ahead of consumption so the
  gather stream never waits on buffers; the last pair is split into two
  single-tile calls so the tail overlaps the final gather,
- finalize reads the PSUM accumulator directly and uses the fast
  approximate reciprocal.

Numerics: bf16 tables + bf16 selector matmul + W_hi-only phase A give
~3.9e-3 relative error vs the 2e-2 budget (validated on HW).
"""

import numpy as np

N_NODES = 20000
N_EDGES = 320000
D = 256
CORES = 8
NPC = N_NODES // CORES          # 2500 nodes per core
NT = (NPC + 127) // 128         # 20 node tiles per core
NROWS = NT * 128                # 2560 padded rows per core
NT_ALL = 160                    # phase-A 128-node tiles over all nodes
NPAD = NT_ALL * 128             # 20480
QT = 4                          # phase-A tiles per PSUM group
HQ = 2                          # hT quads per DMA
GT = 2                          # node-tiles per dma_gather call
BB = 6                          # chunks per DVE mult batch
B0 = 15360                      # Z0 rows (Z is split so gathers from the
                                # first 75%% of nodes start before phase A
                                # finishes the last 25%%)

_cache = {}


def _build(caps0, caps1):
    import concourse.bacc as bacc
    import concourse.mybir as mybir
    from concourse.tile import TileContext

    nc = bacc.Bacc("TRN2")
    f32 = mybir.dt.float32
    bf16 = mybir.dt.bfloat16

    NCH0 = sum(caps0)
    NCH = NCH0 + sum(caps1)             # total chunks across tiles
    NIX = 128 * NCH                     # total gathered edge slots
    G0MAX = max(sum(caps0[t0:t0 + GT]) for t0 in range(0, NT, GT))
    G1MAX = max(sum(caps1[t0:t0 + GT]) for t0 in range(0, NT, GT))
    CTMAX = max(caps0[t] + caps1[t] for t in range(NT))

    hT_d = nc.dram_tensor("hT", [128, 2, NPAD], bf16, kind="ExternalInput")
    WT_d = nc.dram_tensor("WT", [128, 2, D], bf16, kind="ExternalInput")
    Z0_d = nc.dram_tensor("Z0", [B0, 2 * D], bf16, kind="ExternalInput")
    Z1_d = nc.dram_tensor("Z1", [NPAD - B0, 2 * D], bf16, kind="ExternalInput")
    idx_d = nc.dram_tensor("idx", [128, NIX // 16], mybir.dt.int16, kind="ExternalInput")
    S_d = nc.dram_tensor("S", [128, NCH, 128], bf16, kind="ExternalInput")
    hown_d = nc.dram_tensor("hown", [NROWS, D], bf16, kind="ExternalInput")
    out_d = nc.dram_tensor("out", [NROWS, D], bf16, kind="ExternalOutput")

    with TileContext(nc) as tc:
        with (
            tc.tile_pool(name="const", bufs=1) as constp,
            tc.tile_pool(name="pha", bufs=3) as pha,
            tc.tile_pool(name="phb", bufs=3) as phb,
            tc.tile_pool(name="gat", bufs=3) as gat,
            tc.tile_pool(name="wrk", bufs=3) as wrk,
            tc.tile_pool(name="fin", bufs=2) as fin,
            tc.tile_pool(name="psa", bufs=2, space="PSUM") as psa,
            tc.tile_pool(name="psb", bufs=3, space="PSUM") as psb,
        ):
            # ---- constants ----
            WT_sb = constp.tile([128, 2, D], bf16)
            nc.sync.dma_start(WT_sb[:, :, :], WT_d[:, :, :])
            idx_sb = constp.tile([128, NIX // 16], mybir.dt.int16)

            # ---- phase A: e-columns of Z ----
            for i in range(NT_ALL // (QT * HQ)):
                hT_sb = phb.tile([128, 2, QT * HQ * 128], bf16, tag="hT")
                nc.sync.dma_start(
                    hT_sb[:, :, :],
                    hT_d[:, :, i * QT * HQ * 128:(i + 1) * QT * HQ * 128],
                )
                e_sb = pha.tile([128, HQ, QT, D], bf16, tag="es")
                for q in range(HQ):
                    ps = psa.tile([128, QT, D], f32, tag="ps")
                    for u in range(QT):
                        for kb in range(2):
                            nc.tensor.matmul(
                                ps[:, u, :],
                                hT_sb[:, kb, (q * QT + u) * 128:(q * QT + u + 1) * 128],
                                WT_sb[:, kb, :],
                                start=(kb == 0), stop=(kb == 1),
                            )
                    nc.scalar.activation(
                        e_sb[:, q, :, :], ps[:, :, :],
                        mybir.ActivationFunctionType.Exp,
                    )
                for q in range(HQ):
                    r0 = (i * HQ + q) * QT * 128
                    if r0 < B0:
                        zrows = Z0_d[r0:r0 + QT * 128, 0:D]
                    else:
                        zrows = Z1_d[r0 - B0:r0 - B0 + QT * 128, 0:D]
                    nc.sync.dma_start(
                        zrows.rearrange("(u p) c -> p u c", p=128),
                        e_sb[:, q, :, :],
                    )

            # idx for the gathers (loaded on the scalar queue so the sync
            # queue drains the final Z e-writes without queueing behind it)
            nc.scalar.dma_start(idx_sb[:, :], idx_d[:, :])

            # ---- phase B: block-split gathers + segment softmax ----
            # Z is split 75/25: the s0 gathers (src < B0) only depend on the
            # first 15 phase-A blocks, so the gather stream starts ~25%% of
            # phase A early.  s1 gathers trail by PF pairs; consumption of a
            # pair needs both.  All loads ride the scalar HWDGE queue.
            coffs0 = []
            co = 0
            for t in range(NT):
                coffs0.append(co)
                co += caps0[t]
            coffs1 = []
            co = NCH0
            for t in range(NT):
                coffs1.append(co)
                co += caps1[t]
            NP_ = NT // GT
            PF = 2
            zx0s = {}
            zx1s = {}
            S0s = {}
            S1s = {}

            def emit_g(tp, sblk):
                t0 = tp * GT
                if sblk == 0:
                    C01 = sum(caps0[t0:t0 + GT])
                    co0 = coffs0[t0]
                    zx = gat.tile([128, G0MAX, 2 * D], bf16, tag="zx0")
                    S_p = wrk.tile([128, G0MAX, 128], bf16, tag="S0")
                    src_t = Z0_d
                else:
                    C01 = sum(caps1[t0:t0 + GT])
                    co0 = coffs1[t0]
                    zx = gat.tile([128, G1MAX, 2 * D], bf16, tag="zx1")
                    S_p = wrk.tile([128, G1MAX, 128], bf16, tag="S1")
                    src_t = Z1_d
                nc.scalar.dma_start(S_p[:, 0:C01, :], S_d[:, co0:co0 + C01, :])
                nc.gpsimd.dma_gather(
                    zx[:, 0:C01, :], src_t[:, :],
                    idx_sb[:, co0 * 8:(co0 + C01) * 8], 128 * C01, 128 * C01,
                    2 * D, single_packet=False,
                )
                if sblk == 0:
                    zx0s[tp], S0s[tp] = zx, S_p
                else:
                    zx1s[tp], S1s[tp] = zx, S_p

            for tp in range(PF):
                emit_g(tp, 0)

            for tp in range(NP_):
                emit_g(tp, 1)
                if tp + PF < NP_:
                    emit_g(tp + PF, 0)
                zx0, S0_p = zx0s.pop(tp), S0s.pop(tp)
                zx1, S1_p = zx1s.pop(tp), S1s.pop(tp)
                for tt in range(GT):
                    t = tp * GT + tt
                    C0, C1 = caps0[t], caps1[t]
                    C = C0 + C1
                    zo0 = coffs0[t] - coffs0[tp * GT]
                    zo1 = coffs1[t] - coffs1[tp * GT]

                    def src(j):
                        # (S chunk AP, zx chunk AP-pair) for combined index j
                        if j < C0:
                            return S0_p, zx0, zo0 + j
                        return S1_p, zx1, zo1 + (j - C0)

                    ehx = fin.tile([128, CTMAX, D], bf16, tag="ehx")
                    for g in range((C0 + BB - 1) // BB):
                        b = min(BB, C0 - g * BB)
                        nc.vector.tensor_tensor(
                            ehx[:, g * BB:g * BB + b, :],
                            zx0[:, zo0 + g * BB:zo0 + g * BB + b, 0:D],
                            zx0[:, zo0 + g * BB:zo0 + g * BB + b, D:2 * D],
                            mybir.AluOpType.mult,
                        )
                    for g in range((C1 + BB - 1) // BB):
                        b = min(BB, C1 - g * BB)
                        nc.vector.tensor_tensor(
                            ehx[:, C0 + g * BB:C0 + g * BB + b, :],
                            zx1[:, zo1 + g * BB:zo1 + g * BB + b, 0:D],
                            zx1[:, zo1 + g * BB:zo1 + g * BB + b, D:2 * D],
                            mybir.AluOpType.mult,
                        )
                    acc = psb.tile([128, 2 * D], f32, tag="acc")
                    for j in range(C):
                        S_j, zx_j, cj = src(j)
                        nc.tensor.matmul(
                            acc[:, 0:D], S_j[:, cj, :], zx_j[:, cj, 0:D],
                            start=(j == 0), stop=(j == C - 1),
                        )
                    for j in range(C):
                        S_j, zx_j, cj = src(j)
                        nc.tensor.matmul(
                            acc[:, D:2 * D], S_j[:, cj, :], ehx[:, j, :],
                            start=(j == 0), stop=(j == C - 1),
                        )

                    # ---- finalize tile (reads PSUM directly) ----
                    dmax = fin.tile([128, D], f32, tag="dmax")
                    nc.vector.tensor_scalar(
                        dmax[:, :], acc[:, 0:D], 1e-30, None, mybir.AluOpType.max
                    )
                    rec = fin.tile([128, D], f32, tag="rec")
                    nc.vector.reciprocal_approx_fast(rec[:, :], dmax[:, :])
                    mask = fin.tile([128, D], mybir.dt.uint8, tag="mask")
                    nc.vector.tensor_scalar(
                        mask[:, :], acc[:, 0:D], 0.0, None, mybir.AluOpType.is_equal
                    )
                    res = fin.tile([128, D], bf16, tag="res")
                    nc.vector.tensor_tensor(
                        res[:, :], acc[:, D:2 * D], rec[:, :], mybir.AluOpType.mult
                    )
                    hown_sb = fin.tile([128, D], bf16, tag="hown")
                    nc.scalar.dma_start(
                        hown_sb[:, :], hown_d[t * 128:(t + 1) * 128, :]
                    )
                    nc.vector.copy_predicated(res[:, :], mask[:, :], hown_sb[:, :])
                    nc.sync.dma_start(out_d[t * 128:(t + 1) * 128, :], res[:, :])
    nc.compile()
    return nc


def _wrap_idx(ix):
    # dma_gather index layout: logical index i lands at output
    # [partition i%128, slot i//128]; the SBUF index tile stores it at
    # [i%16, 8*(i//128) + (i%128)//16], replicated over the 8 Q7 cores.
    w = ix.astype(np.int16).reshape(-1, 8, 16).transpose(2, 0, 1).reshape(16, -1)
    return np.tile(w, (8, 1))


def kernel(h, W_nb, b_nb, W_self, b_self, src, dst):
    from concourse.bass_utils import run_bass_kernel_spmd
    import ml_dtypes

    bf = ml_dtypes.bfloat16
    h = np.ascontiguousarray(np.asarray(h, dtype=np.float32))
    W = np.asarray(W_self, dtype=np.float32)
    src = np.asarray(src, dtype=np.int64)
    dst = np.asarray(dst, dtype=np.int64)

    order = np.argsort(dst, kind="stable")
    src_s = src[order]
    dst_s = dst[order]

    # per-(core, tile) edge ranges; tiles are 128 consecutive owned nodes
    tile_base = []
    for c in range(CORES):
        for t in range(NT):
            tile_base.append(c * NPC + t * 128)
    bounds_lo = np.searchsorted(dst_s, np.array(tile_base), side="left")
    hi_nodes = [min(b + 128, (b // NPC + 1) * NPC) for b in tile_base]
    bounds_hi = np.searchsorted(dst_s, np.array(hi_nodes), side="left")

    src_lt = (src_s < B0).astype(np.int64)
    cnt0 = np.zeros((CORES, NT), dtype=np.int64)
    cnt1 = np.zeros((CORES, NT), dtype=np.int64)
    for c in range(CORES):
        for t in range(NT):
            i = c * NT + t
            lo, hi = int(bounds_lo[i]), int(bounds_hi[i])
            n0 = int(src_lt[lo:hi].sum())
            cnt0[c, t] = n0
            cnt1[c, t] = (hi - lo) - n0
    caps0 = [int((cnt0[:, t].max() + 127) // 128) for t in range(NT)]
    caps1 = [int(max(cnt1[:, t].max(), 1) + 127) // 128 for t in range(NT)]
    assert max(caps0[t] + caps0[t + 1] for t in range(0, NT, GT)) <= 30, caps0
    NCH0 = sum(caps0)
    NCH = NCH0 + sum(caps1)

    # host-side layout prep
    h_bf = h.astype(bf)
    hT = np.zeros((128, 2, NPAD), dtype=bf)
    hT[:, :, :N_NODES] = np.ascontiguousarray(
        h_bf.T.reshape(2, 128, N_NODES).transpose(1, 0, 2)
    )
    WT = np.ascontiguousarray(
        W.astype(bf).T.reshape(2, 128, D).transpose(1, 0, 2)
    )
    Z0 = np.zeros((B0, 2 * D), dtype=bf)
    Z0[:, D:2 * D] = h_bf[:B0]
    Z1 = np.zeros((NPAD - B0, 2 * D), dtype=bf)
    Z1[:N_NODES - B0, D:2 * D] = h_bf[B0:]

    in_maps = []
    for c in range(CORES):
        idx_parts0 = []
        idx_parts1 = []
        S_all = np.zeros((128, NCH, 128), dtype=bf)
        coff0 = 0
        coff1 = NCH0
        for t in range(NT):
            i = c * NT + t
            lo, hi = int(bounds_lo[i]), int(bounds_hi[i])
            e = src_s[lo:hi]
            dl = dst_s[lo:hi] - tile_base[i]
            m0 = e < B0
            for sel, base, Ct, coff, parts in (
                (m0, 0, caps0[t], coff0, idx_parts0),
                (~m0, B0, caps1[t], coff1, idx_parts1),
            ):
                es = e[sel] - base
                ds = dl[sel]
                n = len(es)
                spad = np.zeros(128 * Ct, dtype=np.int64)
                spad[:n] = es
                parts.append(_wrap_idx(spad))
                ei = np.arange(n)
                S_all[ei % 128, coff + ei // 128, ds] = 1.0
            coff0 += caps0[t]
            coff1 += caps1[t]
        hown = np.zeros((NROWS, D), dtype=bf)
        hown[:NPC] = h_bf[c * NPC:(c + 1) * NPC]
        in_maps.append({
            "hT": hT,
            "WT": WT,
            "Z0": Z0,
            "Z1": Z1,
            "idx": np.ascontiguousarray(
                np.concatenate(idx_parts0 + idx_parts1, axis=1)
            ),
            "S": S_all,
            "hown": hown,
        })

    key = (tuple(caps0), tuple(caps1))
    if key not in _cache:
        _cache[key] = _build(caps0, caps1)
    nc = _cache[key]

    res = run_bass_kernel_spmd(nc, in_maps, core_ids=list(range(CORES)))
    out = np.concatenate(
        [res.results[c]["out"][:NPC] for c in range(CORES)], axis=0
    )
    return out.astype(np.float32)


# revision 26
# speedup vs baseline: 1.0220x; 1.0061x over previous
"""DeepSATConv GNN message-passing kernel for 8 Trainium2 NeuronCores.

Math note: the reference computes a per-channel segment-softmax over
msg = self_h[src] + neib_h[dst].  Within a dst-segment, neib_h[dst] (and
b_self, b_nb) are constant per channel, so they cancel in the softmax.
Hence alpha = segsoftmax(h @ W_self.T) exactly, and
out[n] = segsum(e * h[src]) / segsum(e)  with e = exp((h @ W_self.T)[src]),
falling back to h[n] for zero-in-degree nodes.  W_nb / b_nb / b_self do
not affect the output at all.

Sharding: nodes are split across the 8 cores (2500 each); edges are
partitioned by destination node so segment reductions stay core-local;
h is replicated (the "halo gather" degenerates to replication).

Design notes (evidence from NTFF traces):
- everything feeding the PE is bf16 (4x the fp32 matmul rate),
- the gathered row packs [e | h] bf16 so ONE dma_gather descriptor per
  edge fetches both operands (descriptor GENERATION on the Q7 cores is
  the hard bottleneck: ~6.8 ns/descriptor + ~1.9 us/call, so calls are
  merged two node-tiles at a time),
- Z's h-columns are pre-filled by the host (Z is an ExternalInput); the
  device computes only the e-columns (2 bf16 matmuls per 128-node tile,
  one Exp activation per 4 tiles, batched strided writes),
- the one-hot selector S[e, n] = (dst_local[e] == n) is built on the
  host and DMA'd in bf16 (building it on the DVE made the gather stream
  stall on the DVE's instruction-counter semaphore),
- S loads and gathers are issued PF groups ahead of consumption so the
  gather stream never waits on buffers; Z is split 75/25 (Z0/Z1) so the
  src<15360 gathers start before phase A finishes the last quarter,
- finalize reads the PSUM accumulator directly and uses the fast
  approximate reciprocal.

Numerics: bf16 tables + bf16 selector matmul + W_hi-only phase A give
~3.9e-3 relative error vs the 2e-2 budget (validated on HW).
"""

import numpy as np

N_NODES = 20000
N_EDGES = 320000
D = 256
CORES = 8
NPC = N_NODES // CORES          # 2500 nodes per core
NT = (NPC + 127) // 128         # 20 node tiles per core
NROWS = NT * 128                # 2560 padded rows per core
NT_ALL = 160                    # phase-A 128-node tiles over all nodes
NPAD = NT_ALL * 128             # 20480
QT = 4                          # phase-A tiles per PSUM group
HQ = 2                          # hT quads per DMA
GT = 2                          # node-tiles per dma_gather call
BB = 6                          # chunks per DVE mult batch
B0 = 15360                      # Z0 rows (Z is split so gathers from the
                                # first 75%% of nodes start before phase A
                                # finishes the last 25%%)

_cache = {}


def _build(caps0, caps1):
    import concourse.bacc as bacc
    import concourse.mybir as mybir
    from concourse.tile import TileContext

    nc = bacc.Bacc("TRN2")
    f32 = mybir.dt.float32
    bf16 = mybir.dt.bfloat16

    NCH0 = sum(caps0)
    NCH = NCH0 + sum(caps1)             # total chunks across tiles
    NIX = 128 * NCH                     # total gathered edge slots
    G0MAX = max(sum(caps0[t0:t0 + GT]) for t0 in range(0, NT, GT))
    G1MAX = max(sum(caps1[t0:t0 + GT]) for t0 in range(0, NT, GT))
    CTMAX = max(caps0[t] + caps1[t] for t in range(NT))

    hT_d = nc.dram_tensor("hT", [128, 2, NPAD], bf16, kind="ExternalInput")
    WT_d = nc.dram_tensor("WT", [128, 2, D], bf16, kind="ExternalInput")
    Z0_d = nc.dram_tensor("Z0", [B0, 2 * D], bf16, kind="ExternalInput")
    Z1_d = nc.dram_tensor("Z1", [NPAD - B0, 2 * D], bf16, kind="ExternalInput")
    idx_d = nc.dram_tensor("idx", [128, NIX // 16], mybir.dt.int16, kind="ExternalInput")
    S_d = nc.dram_tensor("S", [128, NCH, 128], bf16, kind="ExternalInput")
    hown_d = nc.dram_tensor("hown", [NROWS, D], bf16, kind="ExternalInput")
    out_d = nc.dram_tensor("out", [NROWS, D], bf16, kind="ExternalOutput")

    with TileContext(nc) as tc:
        with (
            tc.tile_pool(name="const", bufs=1) as constp,
            tc.tile_pool(name="pha", bufs=3) as pha,
            tc.tile_pool(name="phb", bufs=3) as phb,
            tc.tile_pool(name="gat", bufs=3) as gat,
            tc.tile_pool(name="wrk", bufs=3) as wrk,
            tc.tile_pool(name="fin", bufs=2) as fin,
            tc.tile_pool(name="psa", bufs=2, space="PSUM") as psa,
            tc.tile_pool(name="psb", bufs=3, space="PSUM") as psb,
        ):
            # ---- constants ----
            WT_sb = constp.tile([128, 2, D], bf16)
            nc.sync.dma_start(WT_sb[:, :, :], WT_d[:, :, :])
            idx_sb = constp.tile([128, NIX // 16], mybir.dt.int16)

            # ---- phase A: e-columns of Z ----
            for i in range(NT_ALL // (QT * HQ)):
                hT_sb = phb.tile([128, 2, QT * HQ * 128], bf16, tag="hT")
                nc.sync.dma_start(
                    hT_sb[:, :, :],
                    hT_d[:, :, i * QT * HQ * 128:(i + 1) * QT * HQ * 128],
                )
                e_sb = pha.tile([128, HQ, QT, D], bf16, tag="es")
                for q in range(HQ):
                    ps = psa.tile([128, QT, D], f32, tag="ps")
                    for u in range(QT):
                        for kb in range(2):
                            nc.tensor.matmul(
                                ps[:, u, :],
                                hT_sb[:, kb, (q * QT + u) * 128:(q * QT + u + 1) * 128],
                                WT_sb[:, kb, :],
                                start=(kb == 0), stop=(kb == 1),
                            )
                    nc.scalar.activation(
                        e_sb[:, q, :, :], ps[:, :, :],
                        mybir.ActivationFunctionType.Exp,
                    )
                for q in range(HQ):
                    r0 = (i * HQ + q) * QT * 128
                    if r0 < B0:
                        zrows = Z0_d[r0:r0 + QT * 128, 0:D]
                    else:
                        zrows = Z1_d[r0 - B0:r0 - B0 + QT * 128, 0:D]
                    nc.sync.dma_start(
                        zrows.rearrange("(u p) c -> p u c", p=128),
                        e_sb[:, q, :, :],
                    )

            # idx for the gathers (loaded on the scalar queue so the sync
            # queue drains the final Z e-writes without queueing behind it)
            nc.scalar.dma_start(idx_sb[:, :], idx_d[:, :])

            # ---- phase B: block-split gathers + segment softmax ----
            # Z is split 75/25: the s0 gathers (src < B0) only depend on the
            # first 15 phase-A blocks, so the gather stream starts ~25%% of
            # phase A early.  s1 gathers trail by PF pairs; consumption of a
            # pair needs both.  All loads ride the scalar HWDGE queue.
            coffs0 = []
            co = 0
            for t in range(NT):
                coffs0.append(co)
                co += caps0[t]
            coffs1 = []
            co = NCH0
            for t in range(NT):
                coffs1.append(co)
                co += caps1[t]
            NP_ = NT // GT
            PF = 2
            zx0s = {}
            zx1s = {}
            S0s = {}
            S1s = {}

            def emit_g(tp, sblk):
                t0 = tp * GT
                if sblk == 0:
                    C01 = sum(caps0[t0:t0 + GT])
                    co0 = coffs0[t0]
                    zx = gat.tile([128, G0MAX, 2 * D], bf16, tag="zx0")
                    S_p = wrk.tile([128, G0MAX, 128], bf16, tag="S0")
                    src_t = Z0_d
                else:
                    C01 = sum(caps1[t0:t0 + GT])
                    co0 = coffs1[t0]
                    zx = gat.tile([128, G1MAX, 2 * D], bf16, tag="zx1")
                    S_p = wrk.tile([128, G1MAX, 128], bf16, tag="S1")
                    src_t = Z1_d
                nc.scalar.dma_start(S_p[:, 0:C01, :], S_d[:, co0:co0 + C01, :])
                nc.gpsimd.dma_gather(
                    zx[:, 0:C01, :], src_t[:, :],
                    idx_sb[:, co0 * 8:(co0 + C01) * 8], 128 * C01, 128 * C01,
                    2 * D, single_packet=False,
                )
                if sblk == 0:
                    zx0s[tp], S0s[tp] = zx, S_p
                else:
                    zx1s[tp], S1s[tp] = zx, S_p

            for tp in range(PF):
                emit_g(tp, 0)

            for tp in range(NP_):
                emit_g(tp, 1)
                if tp + PF < NP_:
                    emit_g(tp + PF, 0)
                zx0, S0_p = zx0s.pop(tp), S0s.pop(tp)
                zx1, S1_p = zx1s.pop(tp), S1s.pop(tp)
                for tt in range(GT):
                    t = tp * GT + tt
                    C0, C1 = caps0[t], caps1[t]
                    C = C0 + C1
                    zo0 = coffs0[t] - coffs0[tp * GT]
                    zo1 = coffs1[t] - coffs1[tp * GT]

                    def src(j):
                        # (S chunk AP, zx chunk AP-pair) for combined index j
                        if j < C0:
                            return S0_p, zx0, zo0 + j
                        return S1_p, zx1, zo1 + (j - C0)

                    ehx = fin.tile([128, CTMAX, D], bf16, tag="ehx")
                    for g in range((C0 + BB - 1) // BB):
                        b = min(BB, C0 - g * BB)
                        nc.vector.tensor_tensor(
                            ehx[:, g * BB:g * BB + b, :],
                            zx0[:, zo0 + g * BB:zo0 + g * BB + b, 0:D],
                            zx0[:, zo0 + g * BB:zo0 + g * BB + b, D:2 * D],
                            mybir.AluOpType.mult,
                        )
                    for g in range((C1 + BB - 1) // BB):
                        b = min(BB, C1 - g * BB)
                        nc.vector.tensor_tensor(
                            ehx[:, C0 + g * BB:C0 + g * BB + b, :],
                            zx1[:, zo1 + g * BB:zo1 + g * BB + b, 0:D],
                            zx1[:, zo1 + g * BB:zo1 + g * BB + b, D:2 * D],
                            mybir.AluOpType.mult,
                        )
                    acc = psb.tile([128, 2 * D], f32, tag="acc")
                    for j in range(C):
                        S_j, zx_j, cj = src(j)
                        nc.tensor.matmul(
                            acc[:, 0:D], S_j[:, cj, :], zx_j[:, cj, 0:D],
                            start=(j == 0), stop=(j == C - 1),
                        )
                    for j in range(C):
                        S_j, zx_j, cj = src(j)
                        nc.tensor.matmul(
                            acc[:, D:2 * D], S_j[:, cj, :], ehx[:, j, :],
                            start=(j == 0), stop=(j == C - 1),
                        )

                    # ---- finalize tile (reads PSUM directly) ----
                    dmax = fin.tile([128, D], f32, tag="dmax")
                    nc.vector.tensor_scalar(
                        dmax[:, :], acc[:, 0:D], 1e-30, None, mybir.AluOpType.max
                    )
                    rec = fin.tile([128, D], f32, tag="rec")
                    nc.vector.reciprocal_approx_fast(rec[:, :], dmax[:, :])
                    mask = fin.tile([128, D], mybir.dt.uint8, tag="mask")
                    nc.vector.tensor_scalar(
                        mask[:, :], acc[:, 0:D], 0.0, None, mybir.AluOpType.is_equal
                    )
                    res = fin.tile([128, D], bf16, tag="res")
                    nc.vector.tensor_tensor(
                        res[:, :], acc[:, D:2 * D], rec[:, :], mybir.AluOpType.mult
                    )
                    hown_sb = fin.tile([128, D], bf16, tag="hown")
                    nc.scalar.dma_start(
                        hown_sb[:, :], hown_d[t * 128:(t + 1) * 128, :]
                    )
                    nc.vector.copy_predicated(res[:, :], mask[:, :], hown_sb[:, :])
                    nc.sync.dma_start(out_d[t * 128:(t + 1) * 128, :], res[:, :])
    nc.compile()
    return nc


def _wrap_idx(ix):
    # dma_gather index layout: logical index i lands at output
    # [partition i%128, slot i//128]; the SBUF index tile stores it at
    # [i%16, 8*(i//128) + (i%128)//16], replicated over the 8 Q7 cores.
    w = ix.astype(np.int16).reshape(-1, 8, 16).transpose(2, 0, 1).reshape(16, -1)
    return np.tile(w, (8, 1))


def kernel(h, W_nb, b_nb, W_self, b_self, src, dst):
    from concourse.bass_utils import run_bass_kernel_spmd
    import ml_dtypes

    bf = ml_dtypes.bfloat16
    h = np.ascontiguousarray(np.asarray(h, dtype=np.float32))
    W = np.asarray(W_self, dtype=np.float32)
    src = np.asarray(src, dtype=np.int64)
    dst = np.asarray(dst, dtype=np.int64)

    order = np.argsort(dst, kind="stable")
    src_s = src[order]
    dst_s = dst[order]

    # per-(core, tile) edge ranges; tiles are 128 consecutive owned nodes
    tile_base = []
    for c in range(CORES):
        for t in range(NT):
            tile_base.append(c * NPC + t * 128)
    bounds_lo = np.searchsorted(dst_s, np.array(tile_base), side="left")
    hi_nodes = [min(b + 128, (b // NPC + 1) * NPC) for b in tile_base]
    bounds_hi = np.searchsorted(dst_s, np.array(hi_nodes), side="left")

    src_lt = (src_s < B0).astype(np.int64)
    cnt0 = np.zeros((CORES, NT), dtype=np.int64)
    cnt1 = np.zeros((CORES, NT), dtype=np.int64)
    for c in range(CORES):
        for t in range(NT):
            i = c * NT + t
            lo, hi = int(bounds_lo[i]), int(bounds_hi[i])
            n0 = int(src_lt[lo:hi].sum())
            cnt0[c, t] = n0
            cnt1[c, t] = (hi - lo) - n0
    caps0 = [int((cnt0[:, t].max() + 127) // 128) for t in range(NT)]
    caps1 = [int(max(cnt1[:, t].max(), 1) + 127) // 128 for t in range(NT)]
    assert max(caps0[t] + caps0[t + 1] for t in range(0, NT, GT)) <= 30, caps0
    NCH0 = sum(caps0)
    NCH = NCH0 + sum(caps1)

    # host-side layout prep
    h_bf = h.astype(bf)
    hT = np.zeros((128, 2, NPAD), dtype=bf)
    hT[:, :, :N_NODES] = np.ascontiguousarray(
        h_bf.T.reshape(2, 128, N_NODES).transpose(1, 0, 2)
    )
    WT = np.ascontiguousarray(
        W.astype(bf).T.reshape(2, 128, D).transpose(1, 0, 2)
    )
    Z0 = np.zeros((B0, 2 * D), dtype=bf)
    Z0[:, D:2 * D] = h_bf[:B0]
    Z1 = np.zeros((NPAD - B0, 2 * D), dtype=bf)
    Z1[:N_NODES - B0, D:2 * D] = h_bf[B0:]

    in_maps = []
    for c in range(CORES):
        idx_parts0 = []
        idx_parts1 = []
        S_all = np.zeros((128, NCH, 128), dtype=bf)
        coff0 = 0
        coff1 = NCH0
        for t in range(NT):
            i = c * NT + t
            lo, hi = int(bounds_lo[i]), int(bounds_hi[i])
            e = src_s[lo:hi]
            dl = dst_s[lo:hi] - tile_base[i]
            m0 = e < B0
            for sel, base, Ct, coff, parts in (
                (m0, 0, caps0[t], coff0, idx_parts0),
                (~m0, B0, caps1[t], coff1, idx_parts1),
            ):
                es = e[sel] - base
                ds = dl[sel]
                n = len(es)
                spad = np.zeros(128 * Ct, dtype=np.int64)
                spad[:n] = es
                parts.append(_wrap_idx(spad))
                ei = np.arange(n)
                S_all[ei % 128, coff + ei // 128, ds] = 1.0
            coff0 += caps0[t]
            coff1 += caps1[t]
        hown = np.zeros((NROWS, D), dtype=bf)
        hown[:NPC] = h_bf[c * NPC:(c + 1) * NPC]
        in_maps.append({
            "hT": hT,
            "WT": WT,
            "Z0": Z0,
            "Z1": Z1,
            "idx": np.ascontiguousarray(
                np.concatenate(idx_parts0 + idx_parts1, axis=1)
            ),
            "S": S_all,
            "hown": hown,
        })

    key = (tuple(caps0), tuple(caps1))
    if key not in _cache:
        _cache[key] = _build(caps0, caps1)
    nc = _cache[key]

    res = run_bass_kernel_spmd(nc, in_maps, core_ids=list(range(CORES)))
    out = np.concatenate(
        [res.results[c]["out"][:NPC] for c in range(CORES)], axis=0
    )
    return out.astype(np.float32)
